# revision 55
# baseline (speedup 1.0000x reference)
"""Trainium2 Bass kernel for LocalWindowAttention (v2 — transposed-score
dataflow).

Model (reference): B=2, S=4096, D=1024, H=16 heads, hd=64, window W=16
(8 left, 7 right), four dim->dim projections (torch-Linear convention
y = x @ W.T), per-token windowed softmax attention.

Sharding: 8 cores = 2 batches x 4 sequence chunks of 1024 tokens.  Each
core receives a zero-padded halo of 8 left / 7 right tokens (1039 total)
so K/V at chunk boundaries are computed locally - no collectives.

v2 dataflow (all matmuls fp16 operands, fp32 PSUM); query half-blocks of
64 tokens, each attending a 79-key window [t0-8, t0+70]:
  qT/kT = W.T-stationary matmuls in [dout, tok] layout, v natural.
  Per half-block hb (16 per core), per head s:
    scoresT [79 keys, 64 q] = kT_s.T-stat @ qT_s  -- transposed scores,
      with the band mask PRE-WRITTEN into PSUM by a mask matmul
      (maskT x repeated-identity, start=True) so masking costs no
      vector-engine time.  Edge padding is masked the same way (per-core
      mask variants), so no denominator correction is needed.
    expT = Exp(0.125 * scoresT) on ScalarE -> SBUF fp16.
    sums[q] = expT.T-stat @ ones (1-column matmul); rinv = 1/sums (DVE).
    attn[64 q, hd] = expT.T-stat @ v_window  (natural layout, K=79).
    attn_sb = attn * rinv (free-dim broadcast, DVE) -> fp16.
  Per pair of half-blocks (128 tokens): PE-transpose attn -> attnT
  [din, tok], then out = attnT.T-stat @ Wo.T in fp32 PSUM, copied to
  fp16 and DMA'd out (host casts back to fp32).

Head "slots": heads 0-7 use PE base partitions 0:64, heads 8-15 use
64:128 (wq/wk dout blocks are interleaved on the host accordingly) so
each PSUM score bank only ever sees one PE tile position.
"""

import numpy as np

import concourse.bass as bass
import concourse.mybir as mybir
import concourse.tile as tile
from concourse import bacc
from concourse.bass_utils import run_bass_kernel_spmd
from concourse.masks import make_identity

F16 = mybir.dt.float16
F32 = mybir.dt.float32

B, S, D = 2, 4096, 1024
H, HD = 16, 64
WIN, LP, RP = 16, 8, 7
NCORES = 8
CHUNK = S // 4            # tokens per core (1024)
TH = CHUNK + LP + RP      # halo token count (1039)
NHB = CHUNK // 64         # query half-blocks per core (16)
KEYS = 64 + WIN - 1       # keys per half-block window (79)
DT = D // 128             # 128-row tiles across D (8)
NVT = (TH + 127) // 128   # v token tiles (9; last has 15 rows)
VTAIL = TH - 128 * (NVT - 1)  # 15
MASK_NEG = -60000.0       # exactly representable in fp16

TRACE = False             # test.py may set kernel.TRACE = True
DEBUG = False             # adds intermediate-tensor DRAM outputs
LAST_RESULTS = None       # BassKernelResults of the most recent run

_PROGRAM = None


def _build_program():
    """Build + compile the per-core Bass program (cached)."""
    nc = bacc.Bacc("TRN2", target_bir_lowering=False, debug=False)

    xT_d = nc.dram_tensor("xT", [D, TH], F16, kind="ExternalInput")
    wq_d = nc.dram_tensor("wqT", [D, D], F16, kind="ExternalInput")
    wk_d = nc.dram_tensor("wkT", [D, D], F16, kind="ExternalInput")
    wv_d = nc.dram_tensor("wvT", [D, D], F16, kind="ExternalInput")
    wo_d = nc.dram_tensor("woT", [D, D], F16, kind="ExternalInput")
    maskT_d = nc.dram_tensor("maskT", [128, 3, KEYS], F16, kind="ExternalInput")
    i64rep_d = nc.dram_tensor("i64rep", [128, 512], F16, kind="ExternalInput")
    ones_d = nc.dram_tensor("ones", [KEYS, 1], F16, kind="ExternalInput")
    out_d = nc.dram_tensor("out", [CHUNK, D], F16, kind="ExternalOutput")
    if DEBUG:
        dbg_qT_d = nc.dram_tensor("dbg_qT", [128, DT, CHUNK], F16,
                                  kind="ExternalOutput")
        dbg_kT_d = nc.dram_tensor("dbg_kT", [128, DT, TH], F16,
                                  kind="ExternalOutput")
        dbg_v_d = nc.dram_tensor("dbg_v", [128, NVT, D], F16,
                                 kind="ExternalOutput")
        dbg_exp_d = nc.dram_tensor("dbg_exp", [KEYS, 2, 8, 64], F16,
                                   kind="ExternalOutput")
        dbg_attn_d = nc.dram_tensor("dbg_attn", [128, H, HD], F16,
                                    kind="ExternalOutput")
        dbg_attnT_d = nc.dram_tensor("dbg_attnT", [128, DT, CHUNK], F16,
                                     kind="ExternalOutput")

    def msel(hb):
        # mask variant: 0 interior, 1 first half-block, 2 last half-block
        return 1 if hb == 0 else (2 if hb == NHB - 1 else 0)

    with tile.TileContext(nc) as tc:
        with (
            tc.tile_pool(name="const", bufs=1) as cpool,
            tc.tile_pool(name="acts", bufs=1) as apool,
            tc.tile_pool(name="wstream", bufs=2 * DT) as wpool,
            tc.tile_pool(name="soft", bufs=4) as spool,
            tc.tile_pool(name="outsb", bufs=2) as opool,
        ):
            # ---- activations resident in SBUF ----
            xT = apool.tile([128, DT, TH], F16)
            qT = apool.tile([128, DT, CHUNK], F16)
            kT = apool.tile([128, DT, TH], F16)
            v_sb = apool.tile([128, NVT, D], F16)
            vwin = [apool.tile([KEYS, D], F16, name=f"vwin{a}")
                    for a in range(NHB // 2)]
            attnT = apool.tile([128, DT, CHUNK], F16)
            # attention intermediates are jj-major: free index (jj, half, d)
            # puts head s = 8*half + jj at din offset 128*jj + 64*half, the
            # same interleaving the host applies to wq/wk dout and wo din.
            # Pair 0 gets a dedicated buffer: its output projection is
            # deferred to the end of the program (tail has no dependencies).
            attn_sb = [apool.tile([128, 8, 2, HD], F16, name=f"attn{i}")
                       for i in range(3)]
            rinv_sb = apool.tile([128, 2, 8, 2], F32)

            # ---- weight loads ----
            # wq as 8 tile DMAs interleaved with xT tiles: the k-outer qT
            # warmup below starts computing as soon as the first pair lands.
            # wk/wv/wo stream as one big DMA each (less HWDGE overhead).
            wq = []
            for k in range(DT):
                wt = wpool.tile([128, D], F16, tag="w", name=f"w_{k}")
                wsrc = wq_d.ap().rearrange("(j p) o -> p j o", p=128)[:, k]
                xsrc = xT_d.ap().rearrange("(j p) t -> p j t", p=128)[:, k]
                if k == 0:
                    # split so the first qT warmup group can start sooner;
                    # x0's tail columns are only needed by qT-c1/kT, so they
                    # ride behind the (wq1, x1) pair
                    nc.sync.dma_start(wt[:, 0:512], wsrc[:, 0:512])
                    nc.sync.dma_start(xT[:, k, 0:LP + 512], xsrc[:, 0:LP + 512])
                    nc.sync.dma_start(wt[:, 512:D], wsrc[:, 512:D])
                elif k == 1:
                    nc.sync.dma_start(wt, wsrc)
                    nc.sync.dma_start(xT[:, k, :], xsrc)
                    x0src = xT_d.ap().rearrange("(j p) t -> p j t", p=128)[:, 0]
                    nc.sync.dma_start(xT[:, 0, LP + 512:], x0src[:, LP + 512:])
                else:
                    nc.sync.dma_start(wt, wsrc)
                    nc.sync.dma_start(xT[:, k, :], xsrc)
                wq.append(wt)

            # constants (needed only from the attention phase on)
            warmsrc = cpool.tile([128, 128], F16)
            nc.vector.memset(warmsrc, 0.0)
            identity = cpool.tile([128, 128], F16)
            make_identity(nc, identity)
            maskT = cpool.tile([128, 3, KEYS], F16)
            nc.sync.dma_start(maskT, maskT_d.ap())
            i64rep = cpool.tile([128, 512], F16)
            nc.sync.dma_start(i64rep, i64rep_d.ap())
            ones = cpool.tile([KEYS, 1], F16)
            nc.sync.dma_start(ones, ones_d.ap())

            wk_t = apool.tile([128, DT, D], F16)
            nc.sync.dma_start(wk_t, wk_d.ap().rearrange("(j p) o -> p j o", p=128))
            wv_t = apool.tile([128, DT, D], F16)
            nc.sync.dma_start(wv_t, wv_d.ap().rearrange("(j p) o -> p j o", p=128))
            wo_t = apool.tile([128, DT, D], F16)
            nc.sync.dma_start(wo_t, wo_d.ap().rearrange("(j p) o -> p j o", p=128))

            with tc.tile_pool(name="proj_ps", bufs=8, space="PSUM") as proj_ps:
                # PE p-state prewarm: dummy transposes keep the PE
                # continuously busy through the initial DMA wait so the
                # clock-ramp (3us to full speed) starts at ~1us, not ~3us.
                warm = proj_ps.tile([128, 512], F32, tag="proj", name="warm")
                for i in range(25):
                    nc.tensor.matmul(
                        warm[:, 0:128], warmsrc, warmsrc,
                        start=True, stop=True,
                    )

                # ---- qT projection, k-outer in groups of 4 m-tiles so the
                # PE starts as soon as (wq[0], xT[:,0]) arrive ----
                for c0 in (0, 512):
                    for g in (0, 4):
                        pss = [proj_ps.tile([128, 512], F32, tag="proj",
                                            name=f"proj_{c0}_{g}_{i}")
                               for i in range(4)]
                        for k in range(DT):
                            for i in range(4):
                                m = g + i
                                nc.tensor.matmul(
                                    pss[i],
                                    wq[k][:, m * 128:(m + 1) * 128],
                                    xT[:, k, LP + c0: LP + c0 + 512],
                                    start=(k == 0),
                                    stop=(k == DT - 1),
                                )
                        for i in range(4):
                            nc.scalar.activation(
                                qT[:, g + i, c0:c0 + 512], pss[i],
                                mybir.ActivationFunctionType.Copy,
                            )

                # ---- kT projection (k-inner; DMA is ahead by now) ----
                for (c0, cn) in ((0, 512), (512, 512), (1024, TH - 1024)):
                    for m in range(DT):
                        ps = proj_ps.tile([128, 512], F32, tag="proj")
                        for k in range(DT):
                            nc.tensor.matmul(
                                ps[:, :cn],
                                wk_t[:, k, m * 128:(m + 1) * 128],
                                xT[:, k, c0:c0 + cn],
                                start=(k == 0),
                                stop=(k == DT - 1),
                            )
                        nc.vector.tensor_copy(kT[:, m, c0:c0 + cn], ps[:, :cn])

                # ---- v projection, natural [tok, dout]; odd-half-block v
                # windows DMA'd (SBUF->SBUF) as soon as sources are ready ----
                for j in range(NVT):
                    rows = 128 if j < NVT - 1 else VTAIL
                    for n in range(2):
                        ps = proj_ps.tile([128, 512], F32, tag="proj")
                        for k in range(DT):
                            nc.tensor.matmul(
                                ps[:rows, :],
                                xT[:, k, j * 128: j * 128 + rows],
                                wv_t[:, k, n * 512:(n + 1) * 512],
                                start=(k == 0),
                                stop=(k == DT - 1),
                            )
                        nc.vector.tensor_copy(
                            v_sb[:rows, j, n * 512:(n + 1) * 512], ps[:rows, :]
                        )
                    if j >= 1:
                        a = j - 1
                        nc.sync.dma_start(vwin[a][0:64, :], v_sb[64:128, a, :])
                        nc.sync.dma_start(
                            vwin[a][64:KEYS, :], v_sb[0:VTAIL, a + 1, :]
                        )

            # ---- attention + output projection, software-pipelined ----
            with (
                tc.tile_pool(name="score_ps", bufs=3, space="PSUM") as score_ps,
                tc.tile_pool(name="attn_ps", bufs=1, space="PSUM") as attn_ps,
                tc.tile_pool(name="tw_ps", bufs=2, space="PSUM") as tw_ps,
            ):
                av = attn_ps.tile([128, 8, 2, HD], F32, tag="av")
                sums = attn_ps.tile([128, 2, 8, 2], F32, tag="sums")

                expT = {}  # (hb, half) -> exp sbuf tile

                def scores_stage(hb):
                    t0 = 64 * hb
                    for half in (0, 1):
                        l64 = 64 * half
                        sc = score_ps.tile([KEYS, 8, 64], F32, tag="sc")
                        # band mask pre-written into PSUM (one matmul per bank)
                        nc.tensor.matmul(
                            sc,
                            maskT[l64:l64 + 64, msel(hb), :],
                            i64rep[l64:l64 + 64, :],
                            start=True,
                            stop=False,
                        )
                        for jj in range(8):
                            nc.tensor.matmul(
                                sc[:, jj, :],
                                kT[l64:l64 + 64, jj, t0:t0 + KEYS],
                                qT[l64:l64 + 64, jj, t0:t0 + 64],
                                start=False,
                                stop=True,
                            )
                        ex = spool.tile([KEYS, 8, 64], F16, tag="exp")
                        nc.scalar.activation(
                            ex, sc,
                            mybir.ActivationFunctionType.Exp, scale=0.125,
                        )
                        expT[(hb, half)] = ex

                def av_stage(h):
                    par = h % 2
                    buf = (h // 2) % 2
                    abuf = 2 if h < 2 else buf
                    if par == 0:
                        vsrc = v_sb[0:KEYS, h // 2, :]
                    else:
                        vsrc = vwin[h // 2][:, :]
                    for half in (0, 1):
                        ex = expT.pop((h, half))
                        for jj in range(8):
                            s = 8 * half + jj
                            nc.tensor.matmul(
                                av[64 * par:64 * par + 64, jj, half, :],
                                ex[:, jj, :],
                                vsrc[:, 64 * s:64 * s + 64],
                                start=True,
                                stop=True,
                            )
                            nc.tensor.matmul(
                                sums[64 * par:64 * par + 64, buf, jj,
                                     half:half + 1],
                                ex[:, jj, :],
                                ones,
                                start=True,
                                stop=True,
                            )
                    nc.vector.reciprocal(
                        rinv_sb[64 * par:64 * par + 64, buf, :, :],
                        sums[64 * par:64 * par + 64, buf, :, :],
                    )
                    nc.vector.tensor_tensor(
                        attn_sb[abuf][64 * par:64 * par + 64, :, :, :],
                        av[64 * par:64 * par + 64, :, :, :],
                        rinv_sb[64 * par:64 * par + 64, buf, :, :, None]
                        .broadcast_to([64, 8, 2, HD]),
                        mybir.AluOpType.mult,
                    )

                def transp_stage(p, split_copy=False):
                    # pair p = half-blocks (2p, 2p+1) = token block p
                    src = attn_sb[2 if p == 0 else p % 2]
                    tps = tw_ps.tile([128, DT, 128], F16, tag="tw", name="tps")
                    for k in range(DT):
                        nc.tensor.transpose(
                            tps[:, k, :], src[:, k, :, :], identity
                        )
                    if p >= NHB // 2 - 2:
                        # tail pairs: DVE is saturated by the normalize
                        # chain there; copy on the idle ScalarE instead
                        nc.scalar.activation(
                            attnT[:, :, 128 * p:128 * (p + 1)], tps,
                            mybir.ActivationFunctionType.Copy,
                        )
                    else:
                        nc.vector.tensor_copy(
                            attnT[:, :, 128 * p:128 * (p + 1)], tps
                        )

                def transp_half(p, par):
                    # per-parity transpose of one half-block (used for the
                    # final pair so its chain overlaps av of the last hb)
                    src = attn_sb[2 if p == 0 else p % 2]
                    tps = tw_ps.tile([128, DT, 64], F16, tag="tw",
                                     name="tpsh")
                    l64 = 64 * par
                    for k in range(DT):
                        nc.tensor.transpose(
                            tps[:, k, :], src[l64:l64 + 64, k, :, :],
                            identity[l64:l64 + 64, l64:l64 + 64],
                        )
                    nc.scalar.activation(
                        attnT[:, :, 128 * p + l64:128 * p + l64 + 64], tps,
                        mybir.ActivationFunctionType.Copy,
                    )

                def wo_stage(p, nchunks=2, only=None, split_last=False):
                    cw = D // nchunks
                    for n in range(nchunks):
                        if only is not None and n != only:
                            continue
                        ps = tw_ps.tile([128, cw], F32, tag="tw", name="wops")
                        for k in range(DT):
                            nc.tensor.matmul(
                                ps,
                                attnT[:, k, 128 * p:128 * (p + 1)],
                                wo_t[:, k, n * cw:(n + 1) * cw],
                                start=(k == 0),
                                stop=(k == DT - 1),
                            )
                        if split_last and n == nchunks - 1:
                            # final output chunk: copy halves on two engines
                            # in parallel, two pipelined DMAs
                            h = cw // 2
                            osb = opool.tile([128, cw], F16, tag="osb",
                                             name="osb")
                            nc.vector.tensor_copy(osb[:, 0:h], ps[:, 0:h])
                            nc.scalar.activation(
                                osb[:, h:cw], ps[:, h:cw],
                                mybir.ActivationFunctionType.Copy,
                            )
                            base = n * cw
                            for c0 in (0, h):
                                nc.sync.dma_start(
                                    out_d.ap()[128 * p:128 * (p + 1),
                                               base + c0:base + c0 + h],
                                    osb[:, c0:c0 + h],
                                )
                            continue
                        osb = opool.tile([128, cw], F16, tag="osb",
                                         name="osb")
                        if n % 2 == 0:
                            nc.vector.tensor_copy(osb, ps)
                        else:
                            nc.scalar.activation(
                                osb, ps, mybir.ActivationFunctionType.Copy
                            )
                        nc.sync.dma_start(
                            out_d.ap()[128 * p:128 * (p + 1),
                                       n * cw:(n + 1) * cw],
                            osb,
                        )

                DBG_HB = 2  # half-block whose exp/attn pair is dumped

                for hb in range(NHB + 1):
                    if hb < NHB:
                        scores_stage(hb)
                        if DEBUG and hb == DBG_HB:
                            for half in (0, 1):
                                nc.sync.dma_start(
                                    dbg_exp_d.ap()[:, half],
                                    expT[(hb, half)],
                                )
                    if hb >= 1:
                        av_stage(hb - 1)
                        if hb == NHB:
                            # pair 6's last Wo chunk in two halves: the first
                            # is free to hoist into the exp(15) wait; the
                            # second aliases the sums bank so the scheduler
                            # must hold it past av(15) — it then fills the
                            # normalize(15) window.
                            p6 = NHB // 2 - 2
                            wo_stage(p6, nchunks=4, only=3)
                            ps = attn_ps.tile([128, 256], F32, tag="sums",
                                              name="wotail")
                            for k in range(DT):
                                nc.tensor.matmul(
                                    ps,
                                    attnT[:, k, 128 * p6:128 * (p6 + 1)],
                                    wo_t[:, k, 512:768],
                                    start=(k == 0),
                                    stop=(k == DT - 1),
                                )
                            osb = opool.tile([128, 256], F16, tag="osb",
                                             name="osb")
                            nc.scalar.activation(
                                osb, ps, mybir.ActivationFunctionType.Copy
                            )
                            nc.sync.dma_start(
                                out_d.ap()[128 * p6:128 * (p6 + 1), 512:768],
                                osb,
                            )
                            transp_half(NHB // 2 - 1, 1)
                            wo_stage(NHB // 2 - 1)
                        if DEBUG and hb - 1 == DBG_HB + 1:
                            nc.sync.dma_start(
                                dbg_attn_d.ap(),
                                attn_sb[((hb - 1) // 2) % 2],
                            )
                    if hb >= 2 and hb % 2 == 0 and hb < NHB - 2:
                        p = hb // 2 - 1
                        transp_stage(p)
                        wo_stage(p)
                    if hb == NHB - 1:
                        # penultimate pair: transpose + first Wo chunk now,
                        # plus the even half of the final pair
                        p = NHB // 2 - 2
                        transp_stage(p)
                        wo_stage(p, nchunks=2, only=0)
                        transp_half(NHB // 2 - 1, 0)
                if DEBUG:
                    nc.sync.dma_start(dbg_qT_d.ap(), qT)
                    nc.sync.dma_start(dbg_kT_d.ap(), kT)
                    nc.sync.dma_start(dbg_v_d.ap(), v_sb)
                    nc.sync.dma_start(dbg_attnT_d.ap(), attnT)

    nc.compile()
    return nc


def _get_program():
    global _PROGRAM
    if _PROGRAM is None:
        _PROGRAM = _build_program()
    return _PROGRAM


def _band_mask(variant):
    """M[p, c] additive mask for a 79-key x 64-query half-block window."""
    p = np.arange(KEYS)[:, None]
    c = np.arange(64)[None, :]
    valid = (p >= c) & (p <= c + WIN - 1)
    if variant == "left":
        valid &= p >= LP
    elif variant == "right":
        valid &= p < KEYS - RP
    m = np.where(valid, 0.0, MASK_NEG).astype(np.float32)
    return m


def _host_inputs(x, Wq, Wk, Wv, Wo):
    """Shard + preprocess full inputs into per-core input maps."""
    x = np.asarray(x, dtype=np.float32)

    # head-slot permutation for wq/wk: head s dout block -> position
    # 128*(s%8) + 64*(s//8), i.e. heads 0-7 on base partitions 0:64 of
    # each 128-row tile, heads 8-15 on 64:128.
    perm = np.empty(D, np.int64)
    for s in range(H):
        dst = 128 * (s % 8) + 64 * (s // 8)
        perm[dst:dst + 64] = np.arange(64 * s, 64 * s + 64)

    def prep_w(w, permute_cols=False, permute_rows=False):
        wt = np.ascontiguousarray(np.asarray(w, np.float32).T)
        if permute_cols:
            wt = wt[:, perm]
        if permute_rows:
            wt = wt[perm, :]
        return np.ascontiguousarray(wt).astype(np.float16)

    wts = {
        "wqT": prep_w(Wq, permute_cols=True),
        "wkT": prep_w(Wk, permute_cols=True),
        "wvT": prep_w(Wv),
        # attnT din rows come out in slot-interleaved order; permute Wo.T
        # rows to match.
        "woT": prep_w(Wo, permute_rows=True),
    }

    i64rep = np.zeros((128, 512), np.float16)
    for l in range(2):
        for j in range(8):
            i64rep[64 * l:64 * l + 64, 64 * j:64 * j + 64] = np.eye(
                64, dtype=np.float16
            )
    ones = np.ones((KEYS, 1), np.float16)

    m_int = _band_mask(None)
    m_left = _band_mask("left")
    m_right = _band_mask("right")

    in_maps = []
    for cidx in range(NCORES):
        bb, chunk = divmod(cidx, 4)
        g0 = chunk * CHUNK
        lo, hi = g0 - LP, g0 + CHUNK + RP
        xpad = np.zeros((TH, D), np.float32)
        src_lo, src_hi = max(lo, 0), min(hi, S)
        xpad[src_lo - lo: src_hi - lo] = x[bb, src_lo:src_hi]
        xT = np.ascontiguousarray(xpad.T).astype(np.float16)

        # maskT[64l + c, sel, p] = M_sel[p, c], duplicated on both
        # partition halves; sel 1/2 only differ on edge cores.
        m0 = m_int
        m1 = m_left if chunk == 0 else m_int
        m2 = m_right if chunk == 3 else m_int
        maskT = np.zeros((128, 3, KEYS), np.float16)
        for l in range(2):
            for sel, m in enumerate((m0, m1, m2)):
                maskT[64 * l:64 * l + 64, sel, :] = m.T.astype(np.float16)

        in_maps.append(
            {"xT": xT, "maskT": maskT, "i64rep": i64rep, "ones": ones, **wts}
        )
    return in_maps


def kernel(x, Wq, Wk, Wv, Wo):
    global LAST_RESULTS
    nc = _get_program()
    in_maps = _host_inputs(x, Wq, Wk, Wv, Wo)
    res = run_bass_kernel_spmd(
        nc, in_maps, core_ids=list(range(NCORES)), trace=TRACE
    )
    LAST_RESULTS = res
    out = np.empty((B, S, D), np.float32)
    for c in range(NCORES):
        bb, chunk = divmod(c, 4)
        out[bb, chunk * CHUNK:(chunk + 1) * CHUNK] = res.results[c][
            "out"
        ].astype(np.float32)
    return out


# revision 56
# speedup vs baseline: 1.0022x; 1.0022x over previous
"""Trainium2 Bass kernel for LocalWindowAttention (v2 — transposed-score
dataflow).

Model (reference): B=2, S=4096, D=1024, H=16 heads, hd=64, window W=16
(8 left, 7 right), four dim->dim projections (torch-Linear convention
y = x @ W.T), per-token windowed softmax attention.

Sharding: 8 cores = 2 batches x 4 sequence chunks of 1024 tokens.  Each
core receives a zero-padded halo of 8 left / 7 right tokens (1039 total)
so K/V at chunk boundaries are computed locally - no collectives.

v2 dataflow (all matmuls fp16 operands, fp32 PSUM); query half-blocks of
64 tokens, each attending a 79-key window [t0-8, t0+70]:
  qT/kT = W.T-stationary matmuls in [dout, tok] layout, v natural.
  Per half-block hb (16 per core), per head s:
    scoresT [79 keys, 64 q] = kT_s.T-stat @ qT_s  -- transposed scores,
      with the band mask PRE-WRITTEN into PSUM by a mask matmul
      (maskT x repeated-identity, start=True) so masking costs no
      vector-engine time.  Edge padding is masked the same way (per-core
      mask variants), so no denominator correction is needed.
    expT = Exp(0.125 * scoresT) on ScalarE -> SBUF fp16.
    sums[q] = expT.T-stat @ ones (1-column matmul); rinv = 1/sums (DVE).
    attn[64 q, hd] = expT.T-stat @ v_window  (natural layout, K=79).
    attn_sb = attn * rinv (free-dim broadcast, DVE) -> fp16.
  Per pair of half-blocks (128 tokens): PE-transpose attn -> attnT
  [din, tok], then out = attnT.T-stat @ Wo.T in fp32 PSUM, copied to
  fp16 and DMA'd out (host casts back to fp32).

Head "slots": heads 0-7 use PE base partitions 0:64, heads 8-15 use
64:128 (wq/wk dout blocks are interleaved on the host accordingly) so
each PSUM score bank only ever sees one PE tile position.
"""

import numpy as np

import concourse.bass as bass
import concourse.mybir as mybir
import concourse.tile as tile
from concourse import bacc
from concourse.bass_utils import run_bass_kernel_spmd
from concourse.masks import make_identity

F16 = mybir.dt.float16
F32 = mybir.dt.float32

B, S, D = 2, 4096, 1024
H, HD = 16, 64
WIN, LP, RP = 16, 8, 7
NCORES = 8
CHUNK = S // 4            # tokens per core (1024)
TH = CHUNK + LP + RP      # halo token count (1039)
NHB = CHUNK // 64         # query half-blocks per core (16)
KEYS = 64 + WIN - 1       # keys per half-block window (79)
DT = D // 128             # 128-row tiles across D (8)
NVT = (TH + 127) // 128   # v token tiles (9; last has 15 rows)
VTAIL = TH - 128 * (NVT - 1)  # 15
MASK_NEG = -60000.0       # exactly representable in fp16

TRACE = False             # test.py may set kernel.TRACE = True
DEBUG = False             # adds intermediate-tensor DRAM outputs
LAST_RESULTS = None       # BassKernelResults of the most recent run

_PROGRAM = None


def _build_program():
    """Build + compile the per-core Bass program (cached)."""
    nc = bacc.Bacc("TRN2", target_bir_lowering=False, debug=False)

    xT_d = nc.dram_tensor("xT", [D, TH], F16, kind="ExternalInput")
    wq_d = nc.dram_tensor("wqT", [D, D], F16, kind="ExternalInput")
    wk_d = nc.dram_tensor("wkT", [D, D], F16, kind="ExternalInput")
    wv_d = nc.dram_tensor("wvT", [D, D], F16, kind="ExternalInput")
    wo_d = nc.dram_tensor("woT", [D, D], F16, kind="ExternalInput")
    maskT_d = nc.dram_tensor("maskT", [128, 3, KEYS], F16, kind="ExternalInput")
    i64rep_d = nc.dram_tensor("i64rep", [128, 512], F16, kind="ExternalInput")
    ones_d = nc.dram_tensor("ones", [KEYS, 1], F16, kind="ExternalInput")
    out_d = nc.dram_tensor("out", [CHUNK, D], F16, kind="ExternalOutput")
    if DEBUG:
        dbg_qT_d = nc.dram_tensor("dbg_qT", [128, DT, CHUNK], F16,
                                  kind="ExternalOutput")
        dbg_kT_d = nc.dram_tensor("dbg_kT", [128, DT, TH], F16,
                                  kind="ExternalOutput")
        dbg_v_d = nc.dram_tensor("dbg_v", [128, NVT, D], F16,
                                 kind="ExternalOutput")
        dbg_exp_d = nc.dram_tensor("dbg_exp", [KEYS, 2, 8, 64], F16,
                                   kind="ExternalOutput")
        dbg_attn_d = nc.dram_tensor("dbg_attn", [128, H, HD], F16,
                                    kind="ExternalOutput")
        dbg_attnT_d = nc.dram_tensor("dbg_attnT", [128, DT, CHUNK], F16,
                                     kind="ExternalOutput")

    def msel(hb):
        # mask variant: 0 interior, 1 first half-block, 2 last half-block
        return 1 if hb == 0 else (2 if hb == NHB - 1 else 0)

    with tile.TileContext(nc) as tc:
        with (
            tc.tile_pool(name="const", bufs=1) as cpool,
            tc.tile_pool(name="acts", bufs=1) as apool,
            tc.tile_pool(name="wstream", bufs=2 * DT) as wpool,
            tc.tile_pool(name="soft", bufs=4) as spool,
            tc.tile_pool(name="outsb", bufs=2) as opool,
        ):
            # ---- activations resident in SBUF ----
            xT = apool.tile([128, DT, TH], F16)
            qT = apool.tile([128, DT, CHUNK], F16)
            kT = apool.tile([128, DT, TH], F16)
            v_sb = apool.tile([128, NVT, D], F16)
            vwin = [apool.tile([KEYS, D], F16, name=f"vwin{a}")
                    for a in range(NHB // 2)]
            attnT = apool.tile([128, DT, CHUNK], F16)
            # attention intermediates are jj-major: free index (jj, half, d)
            # puts head s = 8*half + jj at din offset 128*jj + 64*half, the
            # same interleaving the host applies to wq/wk dout and wo din.
            # Pair 0 gets a dedicated buffer: its output projection is
            # deferred to the end of the program (tail has no dependencies).
            attn_sb = [apool.tile([128, 8, 2, HD], F16, name=f"attn{i}")
                       for i in range(3)]
            rinv_sb = apool.tile([128, 2, 8, 2], F32)

            # ---- weight loads ----
            # wq as 8 tile DMAs interleaved with xT tiles: the k-outer qT
            # warmup below starts computing as soon as the first pair lands.
            # wk/wv/wo stream as one big DMA each (less HWDGE overhead).
            wq = []
            for k in range(DT):
                wt = wpool.tile([128, D], F16, tag="w", name=f"w_{k}")
                wsrc = wq_d.ap().rearrange("(j p) o -> p j o", p=128)[:, k]
                xsrc = xT_d.ap().rearrange("(j p) t -> p j t", p=128)[:, k]
                if k == 0:
                    # split so the first qT warmup group can start sooner;
                    # x0's tail columns are only needed by qT-c1/kT, so they
                    # ride behind the (wq1, x1) pair
                    nc.sync.dma_start(wt[:, 0:512], wsrc[:, 0:512])
                    nc.sync.dma_start(xT[:, k, 0:LP + 512], xsrc[:, 0:LP + 512])
                    nc.sync.dma_start(wt[:, 512:D], wsrc[:, 512:D])
                elif k == 1:
                    nc.sync.dma_start(wt, wsrc)
                    nc.sync.dma_start(xT[:, k, :], xsrc)
                    x0src = xT_d.ap().rearrange("(j p) t -> p j t", p=128)[:, 0]
                    nc.sync.dma_start(xT[:, 0, LP + 512:], x0src[:, LP + 512:])
                else:
                    nc.sync.dma_start(wt, wsrc)
                    nc.sync.dma_start(xT[:, k, :], xsrc)
                wq.append(wt)

            # constants (needed only from the attention phase on)
            warmsrc = cpool.tile([128, 128], F16)
            nc.vector.memset(warmsrc, 0.0)
            identity = cpool.tile([128, 128], F16)
            make_identity(nc, identity)
            maskT = cpool.tile([128, 3, KEYS], F16)
            nc.sync.dma_start(maskT, maskT_d.ap())
            i64rep = cpool.tile([128, 512], F16)
            nc.sync.dma_start(i64rep, i64rep_d.ap())
            ones = cpool.tile([KEYS, 1], F16)
            nc.sync.dma_start(ones, ones_d.ap())

            wk_t = apool.tile([128, DT, D], F16)
            nc.sync.dma_start(wk_t, wk_d.ap().rearrange("(j p) o -> p j o", p=128))
            wv_t = apool.tile([128, DT, D], F16)
            nc.sync.dma_start(wv_t, wv_d.ap().rearrange("(j p) o -> p j o", p=128))
            wo_t = apool.tile([128, DT, D], F16)
            nc.sync.dma_start(wo_t, wo_d.ap().rearrange("(j p) o -> p j o", p=128))

            with tc.tile_pool(name="proj_ps", bufs=8, space="PSUM") as proj_ps:
                # PE p-state prewarm: dummy transposes keep the PE
                # continuously busy through the initial DMA wait so the
                # clock-ramp (3us to full speed) starts at ~1us, not ~3us.
                warm = proj_ps.tile([128, 512], F32, tag="proj", name="warm")
                for i in range(25):
                    nc.tensor.matmul(
                        warm[:, 0:128], warmsrc, warmsrc,
                        start=True, stop=True,
                    )

                # ---- qT projection, k-outer in groups of 4 m-tiles so the
                # PE starts as soon as (wq[0], xT[:,0]) arrive ----
                for c0 in (0, 512):
                    for g in (0, 4):
                        pss = [proj_ps.tile([128, 512], F32, tag="proj",
                                            name=f"proj_{c0}_{g}_{i}")
                               for i in range(4)]
                        for k in range(DT):
                            for i in range(4):
                                m = g + i
                                nc.tensor.matmul(
                                    pss[i],
                                    wq[k][:, m * 128:(m + 1) * 128],
                                    xT[:, k, LP + c0: LP + c0 + 512],
                                    start=(k == 0),
                                    stop=(k == DT - 1),
                                )
                        for i in range(4):
                            nc.scalar.activation(
                                qT[:, g + i, c0:c0 + 512], pss[i],
                                mybir.ActivationFunctionType.Copy,
                            )

                # ---- kT projection (k-inner; DMA is ahead by now) ----
                for (c0, cn) in ((0, 512), (512, 512), (1024, TH - 1024)):
                    for m in range(DT):
                        ps = proj_ps.tile([128, 512], F32, tag="proj")
                        for k in range(DT):
                            nc.tensor.matmul(
                                ps[:, :cn],
                                wk_t[:, k, m * 128:(m + 1) * 128],
                                xT[:, k, c0:c0 + cn],
                                start=(k == 0),
                                stop=(k == DT - 1),
                            )
                        nc.vector.tensor_copy(kT[:, m, c0:c0 + cn], ps[:, :cn])

                # ---- v projection, natural [tok, dout]; odd-half-block v
                # windows DMA'd (SBUF->SBUF) as soon as sources are ready ----
                for j in range(NVT):
                    rows = 128 if j < NVT - 1 else VTAIL
                    for n in range(2):
                        ps = proj_ps.tile([128, 512], F32, tag="proj")
                        for k in range(DT):
                            nc.tensor.matmul(
                                ps[:rows, :],
                                xT[:, k, j * 128: j * 128 + rows],
                                wv_t[:, k, n * 512:(n + 1) * 512],
                                start=(k == 0),
                                stop=(k == DT - 1),
                            )
                        nc.vector.tensor_copy(
                            v_sb[:rows, j, n * 512:(n + 1) * 512], ps[:rows, :]
                        )
                    if j >= 1:
                        a = j - 1
                        nc.sync.dma_start(vwin[a][0:64, :], v_sb[64:128, a, :])
                        nc.sync.dma_start(
                            vwin[a][64:KEYS, :], v_sb[0:VTAIL, a + 1, :]
                        )

            # ---- attention + output projection, software-pipelined ----
            with (
                tc.tile_pool(name="score_ps", bufs=3, space="PSUM") as score_ps,
                tc.tile_pool(name="attn_ps", bufs=1, space="PSUM") as attn_ps,
                tc.tile_pool(name="tw_ps", bufs=2, space="PSUM") as tw_ps,
            ):
                av = attn_ps.tile([128, 8, 2, HD], F32, tag="av")
                sums = attn_ps.tile([128, 2, 8, 2], F32, tag="sums")

                expT = {}  # (hb, half) -> exp sbuf tile

                def scores_stage(hb):
                    t0 = 64 * hb
                    for half in (0, 1):
                        l64 = 64 * half
                        sc = score_ps.tile([KEYS, 8, 64], F32, tag="sc")
                        # band mask pre-written into PSUM (one matmul per bank)
                        nc.tensor.matmul(
                            sc,
                            maskT[l64:l64 + 64, msel(hb), :],
                            i64rep[l64:l64 + 64, :],
                            start=True,
                            stop=False,
                        )
                        for jj in range(8):
                            nc.tensor.matmul(
                                sc[:, jj, :],
                                kT[l64:l64 + 64, jj, t0:t0 + KEYS],
                                qT[l64:l64 + 64, jj, t0:t0 + 64],
                                start=False,
                                stop=True,
                            )
                        ex = spool.tile([KEYS, 8, 64], F16, tag="exp")
                        nc.scalar.activation(
                            ex, sc,
                            mybir.ActivationFunctionType.Exp, scale=0.125,
                        )
                        expT[(hb, half)] = ex

                def av_stage(h):
                    par = h % 2
                    buf = (h // 2) % 2
                    abuf = 2 if h < 2 else buf
                    if par == 0:
                        vsrc = v_sb[0:KEYS, h // 2, :]
                    else:
                        vsrc = vwin[h // 2][:, :]
                    exs = {half: expT.pop((h, half)) for half in (0, 1)}
                    # all sums first: the reciprocal then overlaps the AV
                    # matmuls instead of waiting for the whole block
                    for half in (0, 1):
                        for jj in range(8):
                            nc.tensor.matmul(
                                sums[64 * par:64 * par + 64, buf, jj,
                                     half:half + 1],
                                exs[half][:, jj, :],
                                ones,
                                start=True,
                                stop=True,
                            )
                    for half in (0, 1):
                        for jj in range(8):
                            s = 8 * half + jj
                            nc.tensor.matmul(
                                av[64 * par:64 * par + 64, jj, half, :],
                                exs[half][:, jj, :],
                                vsrc[:, 64 * s:64 * s + 64],
                                start=True,
                                stop=True,
                            )
                    nc.vector.reciprocal(
                        rinv_sb[64 * par:64 * par + 64, buf, :, :],
                        sums[64 * par:64 * par + 64, buf, :, :],
                    )
                    nc.vector.tensor_tensor(
                        attn_sb[abuf][64 * par:64 * par + 64, :, :, :],
                        av[64 * par:64 * par + 64, :, :, :],
                        rinv_sb[64 * par:64 * par + 64, buf, :, :, None]
                        .broadcast_to([64, 8, 2, HD]),
                        mybir.AluOpType.mult,
                    )

                def transp_stage(p, split_copy=False):
                    # pair p = half-blocks (2p, 2p+1) = token block p
                    src = attn_sb[2 if p == 0 else p % 2]
                    tps = tw_ps.tile([128, DT, 128], F16, tag="tw", name="tps")
                    for k in range(DT):
                        nc.tensor.transpose(
                            tps[:, k, :], src[:, k, :, :], identity
                        )
                    if p >= NHB // 2 - 2:
                        # tail pairs: DVE is saturated by the normalize
                        # chain there; copy on the idle ScalarE instead
                        nc.scalar.activation(
                            attnT[:, :, 128 * p:128 * (p + 1)], tps,
                            mybir.ActivationFunctionType.Copy,
                        )
                    else:
                        nc.vector.tensor_copy(
                            attnT[:, :, 128 * p:128 * (p + 1)], tps
                        )

                def transp_half(p, par):
                    # per-parity transpose of one half-block (used for the
                    # final pair so its chain overlaps av of the last hb)
                    src = attn_sb[2 if p == 0 else p % 2]
                    tps = tw_ps.tile([128, DT, 64], F16, tag="tw",
                                     name="tpsh")
                    l64 = 64 * par
                    for k in range(DT):
                        nc.tensor.transpose(
                            tps[:, k, :], src[l64:l64 + 64, k, :, :],
                            identity[l64:l64 + 64, l64:l64 + 64],
                        )
                    nc.scalar.activation(
                        attnT[:, :, 128 * p + l64:128 * p + l64 + 64], tps,
                        mybir.ActivationFunctionType.Copy,
                    )

                def wo_stage(p, nchunks=2, only=None, split_last=False):
                    cw = D // nchunks
                    for n in range(nchunks):
                        if only is not None and n != only:
                            continue
                        ps = tw_ps.tile([128, cw], F32, tag="tw", name="wops")
                        for k in range(DT):
                            nc.tensor.matmul(
                                ps,
                                attnT[:, k, 128 * p:128 * (p + 1)],
                                wo_t[:, k, n * cw:(n + 1) * cw],
                                start=(k == 0),
                                stop=(k == DT - 1),
                            )
                        if split_last and n == nchunks - 1:
                            # final output chunk: copy halves on two engines
                            # in parallel, two pipelined DMAs
                            h = cw // 2
                            osb = opool.tile([128, cw], F16, tag="osb",
                                             name="osb")
                            nc.vector.tensor_copy(osb[:, 0:h], ps[:, 0:h])
                            nc.scalar.activation(
                                osb[:, h:cw], ps[:, h:cw],
                                mybir.ActivationFunctionType.Copy,
                            )
                            base = n * cw
                            for c0 in (0, h):
                                nc.sync.dma_start(
                                    out_d.ap()[128 * p:128 * (p + 1),
                                               base + c0:base + c0 + h],
                                    osb[:, c0:c0 + h],
                                )
                            continue
                        osb = opool.tile([128, cw], F16, tag="osb",
                                         name="osb")
                        if n % 2 == 0:
                            nc.vector.tensor_copy(osb, ps)
                        else:
                            nc.scalar.activation(
                                osb, ps, mybir.ActivationFunctionType.Copy
                            )
                        nc.sync.dma_start(
                            out_d.ap()[128 * p:128 * (p + 1),
                                       n * cw:(n + 1) * cw],
                            osb,
                        )

                DBG_HB = 2  # half-block whose exp/attn pair is dumped

                for hb in range(NHB + 1):
                    if hb < NHB:
                        scores_stage(hb)
                        if DEBUG and hb == DBG_HB:
                            for half in (0, 1):
                                nc.sync.dma_start(
                                    dbg_exp_d.ap()[:, half],
                                    expT[(hb, half)],
                                )
                    if hb >= 1:
                        av_stage(hb - 1)
                        if hb == NHB:
                            # pair 6's last Wo chunk in two halves: the first
                            # is free to hoist into the exp(15) wait; the
                            # second aliases the sums bank so the scheduler
                            # must hold it past av(15) — it then fills the
                            # normalize(15) window.
                            p6 = NHB // 2 - 2
                            wo_stage(p6, nchunks=4, only=3)
                            ps = attn_ps.tile([128, 256], F32, tag="sums",
                                              name="wotail")
                            for k in range(DT):
                                nc.tensor.matmul(
                                    ps,
                                    attnT[:, k, 128 * p6:128 * (p6 + 1)],
                                    wo_t[:, k, 512:768],
                                    start=(k == 0),
                                    stop=(k == DT - 1),
                                )
                            osb = opool.tile([128, 256], F16, tag="osb",
                                             name="osb")
                            nc.scalar.activation(
                                osb, ps, mybir.ActivationFunctionType.Copy
                            )
                            nc.sync.dma_start(
                                out_d.ap()[128 * p6:128 * (p6 + 1), 512:768],
                                osb,
                            )
                            transp_half(NHB // 2 - 1, 1)
                            wo_stage(NHB // 2 - 1)
                        if DEBUG and hb - 1 == DBG_HB + 1:
                            nc.sync.dma_start(
                                dbg_attn_d.ap(),
                                attn_sb[((hb - 1) // 2) % 2],
                            )
                    if hb >= 2 and hb % 2 == 0 and hb < NHB - 2:
                        p = hb // 2 - 1
                        transp_stage(p)
                        wo_stage(p)
                    if hb == NHB - 1:
                        # penultimate pair: transpose + first Wo chunk now,
                        # plus the even half of the final pair
                        p = NHB // 2 - 2
                        transp_stage(p)
                        wo_stage(p, nchunks=2, only=0)
                        transp_half(NHB // 2 - 1, 0)
                if DEBUG:
                    nc.sync.dma_start(dbg_qT_d.ap(), qT)
                    nc.sync.dma_start(dbg_kT_d.ap(), kT)
                    nc.sync.dma_start(dbg_v_d.ap(), v_sb)
                    nc.sync.dma_start(dbg_attnT_d.ap(), attnT)

    nc.compile()
    return nc


def _get_program():
    global _PROGRAM
    if _PROGRAM is None:
        _PROGRAM = _build_program()
    return _PROGRAM


def _band_mask(variant):
    """M[p, c] additive mask for a 79-key x 64-query half-block window."""
    p = np.arange(KEYS)[:, None]
    c = np.arange(64)[None, :]
    valid = (p >= c) & (p <= c + WIN - 1)
    if variant == "left":
        valid &= p >= LP
    elif variant == "right":
        valid &= p < KEYS - RP
    m = np.where(valid, 0.0, MASK_NEG).astype(np.float32)
    return m


def _host_inputs(x, Wq, Wk, Wv, Wo):
    """Shard + preprocess full inputs into per-core input maps."""
    x = np.asarray(x, dtype=np.float32)

    # head-slot permutation for wq/wk: head s dout block -> position
    # 128*(s%8) + 64*(s//8), i.e. heads 0-7 on base partitions 0:64 of
    # each 128-row tile, heads 8-15 on 64:128.
    perm = np.empty(D, np.int64)
    for s in range(H):
        dst = 128 * (s % 8) + 64 * (s // 8)
        perm[dst:dst + 64] = np.arange(64 * s, 64 * s + 64)

    def prep_w(w, permute_cols=False, permute_rows=False):
        wt = np.ascontiguousarray(np.asarray(w, np.float32).T)
        if permute_cols:
            wt = wt[:, perm]
        if permute_rows:
            wt = wt[perm, :]
        return np.ascontiguousarray(wt).astype(np.float16)

    wts = {
        "wqT": prep_w(Wq, permute_cols=True),
        "wkT": prep_w(Wk, permute_cols=True),
        "wvT": prep_w(Wv),
        # attnT din rows come out in slot-interleaved order; permute Wo.T
        # rows to match.
        "woT": prep_w(Wo, permute_rows=True),
    }

    i64rep = np.zeros((128, 512), np.float16)
    for l in range(2):
        for j in range(8):
            i64rep[64 * l:64 * l + 64, 64 * j:64 * j + 64] = np.eye(
                64, dtype=np.float16
            )
    ones = np.ones((KEYS, 1), np.float16)

    m_int = _band_mask(None)
    m_left = _band_mask("left")
    m_right = _band_mask("right")

    in_maps = []
    for cidx in range(NCORES):
        bb, chunk = divmod(cidx, 4)
        g0 = chunk * CHUNK
        lo, hi = g0 - LP, g0 + CHUNK + RP
        xpad = np.zeros((TH, D), np.float32)
        src_lo, src_hi = max(lo, 0), min(hi, S)
        xpad[src_lo - lo: src_hi - lo] = x[bb, src_lo:src_hi]
        xT = np.ascontiguousarray(xpad.T).astype(np.float16)

        # maskT[64l + c, sel, p] = M_sel[p, c], duplicated on both
        # partition halves; sel 1/2 only differ on edge cores.
        m0 = m_int
        m1 = m_left if chunk == 0 else m_int
        m2 = m_right if chunk == 3 else m_int
        maskT = np.zeros((128, 3, KEYS), np.float16)
        for l in range(2):
            for sel, m in enumerate((m0, m1, m2)):
                maskT[64 * l:64 * l + 64, sel, :] = m.T.astype(np.float16)

        in_maps.append(
            {"xT": xT, "maskT": maskT, "i64rep": i64rep, "ones": ones, **wts}
        )
    return in_maps


def kernel(x, Wq, Wk, Wv, Wo):
    global LAST_RESULTS
    nc = _get_program()
    in_maps = _host_inputs(x, Wq, Wk, Wv, Wo)
    res = run_bass_kernel_spmd(
        nc, in_maps, core_ids=list(range(NCORES)), trace=TRACE
    )
    LAST_RESULTS = res
    out = np.empty((B, S, D), np.float32)
    for c in range(NCORES):
        bb, chunk = divmod(c, 4)
        out[bb, chunk * CHUNK:(chunk + 1) * CHUNK] = res.results[c][
            "out"
        ].astype(np.float32)
    return out
